# revision 20
# baseline (speedup 1.0000x reference)
"""GNN message-passing kernel for TRN2, 8 NeuronCores (v2).

Strategy (edge/destination-row parallelism, fp16 data path):
- Rows (destination nodes) are split into 8 contiguous per-core ranges, each
  range into 49 blocks of 128 rows. Edges go to the core/block owning their
  (sorted) destination row, padded per block to whole 128-edge tiles.
- Node features live in a fp16 table T = [x(256) | y(64) | r2x | pad]
  (644B rows) replicated per core in DRAM, gathered one row per partition
  per indirect DMA. NOTE: multi-index-per-partition indirect_dma_start is
  CoreSim-only — real HW DGE consumes a single offset per partition and
  streams the rest contiguously from the first row. Batching gathers needs
  InstDMAGatherAnt (256B-aligned rows, int16 indices, negative=skip) and
  is the main remaining optimization (Pool descgen is ~1us/tile here).
- Per edge tile: rg = S @ R' on the PE (full row, incl. the r2x norm
  column); y section becomes diff via a -I @ g_y accumulate. pw_x comes
  from the norm identity r2_l + r2_c - 2*dot with dot via one DVE
  scalar_tensor_tensor multiply-accumulate (g from SBUF, rg from PSUM —
  only one PSUM operand is legal); pw_y via one ACT Square-accumulate on
  the diff. w = exp(-pw/sigma^2) on ACT (batched over KM tiles); weighted
  selection matrices S*w built in one fused tensor_scalar
  (is_equal + mult) per part; aggregation + degree via PE matmuls into a
  per-block PSUM accumulator (one accumulation-group start/stop per PSUM
  zero-region; degree uses a constant ones column as rhs).
- Block epilogue: oxy = agg * (1/deg) via ACT Copy-with-scale, contribution
  to out = concat(feats) @ W via DMA-transposed feature chunks (no PE
  transposes, no PSUM round trips).
- Pass A writes next-level features (t1); an AllGather shares them between
  the two passes, chunked 7x and issued as soon as each 7-block chunk of
  pass A finishes so it overlaps with remaining pass-A compute.
"""
import sys

if '/opt/trn_rl_repo' not in sys.path:
    sys.path.insert(0, '/opt/trn_rl_repo')

import numpy as np

N_NODES = 50000
N_EDGES = 800000
DX, DY = 256, 64
DROW = DX + DY            # feature cols (x|y) = 320
TW = DROW + 2             # table row width: [x(256)|y(64)|r2x|pad] = 322
P = 128
NCORES = 8
BPC = 49                  # blocks per core
RPC = BPC * P             # rows per core = 6272
N_PAD = NCORES * RPC      # 50176
NCH = 7                   # AllGather chunks per pass
CHROWS = RPC // NCH       # 896 rows per core per chunk
CHFULL = CHROWS * NCORES  # 7168 rows per chunk in t1_full
KG = 1                    # one index per partition per gather (HW DGE limit)
KM = 4                    # tiles per mask/exp group

LAST_RESULT = None
LAST_NC = None
LAST_IN_MAPS = None


def _build_program(tpb, sig_scales, with_collective=True):
    """tpb: list[int] tiles per block (same for every core); sig_scales:
    (sxA, syA, sxB, syB) = -1/sigma^2 floats baked as immediates."""
    from concourse import bacc, bass, mybir
    import concourse.tile as tile

    f32 = mybir.dt.float32
    f16 = mybir.dt.float16
    i32 = mybir.dt.int32
    T_tiles = sum(tpb)
    off = np.concatenate([[0], np.cumsum(tpb)]).astype(int)
    sxA, syA, sxB, syB = sig_scales

    nc = bacc.Bacc(num_devices=NCORES, dynamic_dma_scratch_size=65536)
    t0_full = nc.dram_tensor("t0_full", [N_PAD, TW], f16, kind="ExternalInput")
    t0_local = nc.dram_tensor("t0_local", [RPC, TW], f16, kind="ExternalInput")
    colrA = nc.dram_tensor("colrA", [P, T_tiles], i32, kind="ExternalInput")
    colrB = nc.dram_tensor("colrB", [P, T_tiles], i32, kind="ExternalInput")
    rowlf = nc.dram_tensor("rowlf", [P, T_tiles], f32, kind="ExternalInput")
    rowlT_d = nc.dram_tensor("rowlT_d", [T_tiles, P], f16, kind="ExternalInput")
    iota_in = nc.dram_tensor("iota", [P, P], f16, kind="ExternalInput")
    iotaP_in = nc.dram_tensor("iotaP", [P, KM * P], f16, kind="ExternalInput")
    negI_in = nc.dram_tensor("negI", [P, P], f16, kind="ExternalInput")
    w_mat = nc.dram_tensor("w_mat", [896, 64], f16, kind="ExternalInput")
    out_d = nc.dram_tensor("out", [RPC, 64], f32, kind="ExternalOutput")

    with tile.TileContext(nc) as tc:
        with (
            tc.tile_pool(name="const", bufs=1) as cst,
            tc.tile_pool(name="persist", bufs=1) as prs,
            tc.tile_pool(name="blk", bufs=3) as blk,
            tc.tile_pool(name="gat", bufs=4) as gp,
            tc.tile_pool(name="msk", bufs=3) as mp,
            tc.tile_pool(name="sw", bufs=4) as swp,
            tc.tile_pool(name="jk", bufs=3) as jk,
            tc.tile_pool(name="epi", bufs=3) as ep,
            tc.tile_pool(name="ps_agg", bufs=2, space="PSUM") as ps_agg,
            tc.tile_pool(name="ps_rg", bufs=4, space="PSUM") as ps_rg,
            tc.tile_pool(name="ps_out", bufs=2, space="PSUM") as ps_o,
            tc.tile_pool(name="dram", bufs=1, space="DRAM") as dr,
        ):
            iota_t = cst.tile([P, P], f16, tag="iota")
            nc.sync.dma_start(out=iota_t[:], in_=iota_in[:, :])
            iotaP_t = cst.tile([P, KM * P], f16, tag="iotaP")
            nc.sync.dma_start(out=iotaP_t[:], in_=iotaP_in[:, :])
            negI = cst.tile([P, P], f16, tag="negI")
            nc.sync.dma_start(out=negI[:], in_=negI_in[:, :])
            ones_c = cst.tile([P, 1], f16, tag="ones")
            nc.vector.memset(ones_c[:], 1.0)
            # W chunks: rows of W on partitions (feats order x0,x1,x2,y1,y2)
            wt = []
            for i in range(6):
                t = cst.tile([P, 64], f16, tag=f"wt{i}")
                nc.sync.dma_start(out=t[:], in_=w_mat[i * 128:(i + 1) * 128, :])
                wt.append(t)
            wy = []
            for i in range(2):
                t = cst.tile([64, 64], f16, tag=f"wy{i}")
                nc.sync.dma_start(out=t[:], in_=w_mat[768 + i * 64: 768 + (i + 1) * 64, :])
                wy.append(t)

            out_acc = prs.tile([P, BPC * 64], f32, tag="oacc")

            t1_slice = dr.tile([RPC, TW], f16, tag="t1s")
            t1_full = dr.tile([N_PAD, TW], f16, tag="t1f")

            for pas in range(2):
                table = t0_full if pas == 0 else t1_full
                colr = colrA if pas == 0 else colrB
                sx, sy = (sxA, syA) if pas == 0 else (sxB, syB)
                for b in range(BPC):
                    ntl = tpb[b]
                    rp = blk.tile([P, TW], f16, tag="rp")
                    if pas == 0:
                        nc.sync.dma_start(out=rp[:], in_=t0_local[b * P:(b + 1) * P, :])
                    else:
                        nc.sync.dma_start(out=rp[:], in_=t1_slice[b * P:(b + 1) * P, :])
                    col_sb = blk.tile([P, ntl], i32, tag="col")
                    nc.sync.dma_start(out=col_sb[:], in_=colr[:, off[b]:off[b] + ntl])
                    rowl_sb = blk.tile([P, ntl], f32, tag="rowl")
                    nc.sync.dma_start(out=rowl_sb[:], in_=rowlf[:, off[b]:off[b] + ntl])

                    agg = ps_agg.tile([P, DROW + 2], f32, tag="agg")

                    # batched gathers
                    g_tiles = []
                    for j0 in range(0, ntl, KG):
                        kgi = min(KG, ntl - j0)
                        gt = gp.tile([P, KG * TW], f16, tag="g")
                        nc.gpsimd.indirect_dma_start(
                            out=gt[:, 0:kgi * TW], out_offset=None,
                            in_=table[:, :],
                            in_offset=bass.IndirectOffsetOnAxis(
                                ap=col_sb[:, j0:j0 + kgi], axis=0),
                        )
                        g_tiles.append(gt)

                    for m0 in range(0, ntl, KM):
                        kmi = min(KM, ntl - m0)
                        # replicated per-tile dest rows: [p, t*128+e] = rowl[t][e]
                        rlT = mp.tile([P, KM * P], f16, tag="rlT")
                        nc.sync.dma_start(
                            out=rlT[:, 0:kmi * P],
                            in_=rowlT_d[off[b] + m0: off[b] + m0 + kmi, :]
                                .unsqueeze(0).to_broadcast([P, kmi, P]))
                        sT_all = mp.tile([P, KM * P], f16, tag="sT")
                        nc.vector.tensor_tensor(
                            out=sT_all[:, 0:kmi * P], in0=iotaP_t[:, 0:kmi * P],
                            in1=rlT[:, 0:kmi * P], op=mybir.AluOpType.is_equal)

                        pw = mp.tile([P, 2 * KM], f32, tag="pw")
                        w_all = mp.tile([P, 2 * KM], f32, tag="w")
                        dtmp = mp.tile([P, KM], f32, tag="dt")
                        s2t = mp.tile([P, KM], f32, tag="s2")
                        for dt in range(kmi):
                            t = m0 + dt
                            gsl = g_tiles[t // KG][:, (t % KG) * TW:
                                                   (t % KG) * TW + TW]
                            rg = ps_rg.tile([P, TW], f32, tag="rg")
                            sTl = sT_all[:, dt * P:(dt + 1) * P]
                            # rg = S @ R' over the full row (x, y, r2x, pad);
                            # then y section becomes diff via -I @ g_y
                            nc.tensor.matmul(out=rg[:], lhsT=sTl,
                                             rhs=rp[:], start=True, stop=False)
                            nc.tensor.matmul(out=rg[:, DX:DROW], lhsT=negI[:],
                                             rhs=gsl[:, DX:DROW],
                                             start=False, stop=True)
                            # dot_x = sum(g_x * rg_x); pw_x = r2x + g2x - 2 dot_x
                            jx = jk.tile([P, DX], f16, tag="jx")
                            dtc = dtmp[:, dt:dt + 1]
                            nc.vector.scalar_tensor_tensor(
                                out=jx[:], in0=gsl[:, 0:DX], scalar=1.0,
                                in1=rg[:, 0:DX], op0=mybir.AluOpType.mult,
                                op1=mybir.AluOpType.mult, accum_out=dtc)
                            s2c = s2t[:, dt:dt + 1]
                            nc.vector.tensor_tensor(
                                out=s2c, in0=rg[:, DROW:DROW + 1],
                                in1=gsl[:, DROW:DROW + 1], op=mybir.AluOpType.add)
                            nc.vector.tensor_scalar(
                                out=pw[:, dt:dt + 1], in0=dtc, scalar1=-2.0,
                                scalar2=s2c, op0=mybir.AluOpType.mult,
                                op1=mybir.AluOpType.add)
                            jy = jk.tile([P, DY], f16, tag="jy")
                            nc.scalar.activation(
                                out=jy[:], in_=rg[:, DX:DROW],
                                func=mybir.ActivationFunctionType.Square,
                                accum_out=pw[:, KM + dt:KM + dt + 1])
                        if sx == sy and kmi == KM:
                            nc.scalar.activation(
                                out=w_all[:], in_=pw[:],
                                func=mybir.ActivationFunctionType.Exp, scale=sx)
                        else:
                            nc.scalar.activation(
                                out=w_all[:, 0:kmi], in_=pw[:, 0:kmi],
                                func=mybir.ActivationFunctionType.Exp, scale=sx)
                            nc.scalar.activation(
                                out=w_all[:, KM:KM + kmi], in_=pw[:, KM:KM + kmi],
                                func=mybir.ActivationFunctionType.Exp, scale=sy)
                        for dt in range(kmi):
                            t = m0 + dt
                            gsl = g_tiles[t // KG][:, (t % KG) * TW:
                                                   (t % KG) * TW + TW]
                            sxw = swp.tile([P, P], f16, tag="sxw")
                            nc.vector.tensor_scalar(
                                out=sxw[:], in0=iota_t[:],
                                scalar1=rowl_sb[:, t:t + 1],
                                scalar2=w_all[:, dt:dt + 1],
                                op0=mybir.AluOpType.is_equal,
                                op1=mybir.AluOpType.mult)
                            syw = swp.tile([P, P], f16, tag="syw")
                            nc.vector.tensor_scalar(
                                out=syw[:], in0=iota_t[:],
                                scalar1=rowl_sb[:, t:t + 1],
                                scalar2=w_all[:, KM + dt:KM + dt + 1],
                                op0=mybir.AluOpType.is_equal,
                                op1=mybir.AluOpType.mult)
                            first, last = (t == 0), (t == ntl - 1)
                            # one accumulation group per PSUM zero-region:
                            # only the very first matmul starts it, only the
                            # very last stops it
                            nc.tensor.matmul(out=agg[:, 0:DX], lhsT=sxw[:],
                                             rhs=gsl[:, 0:DX],
                                             start=first, stop=False)
                            nc.tensor.matmul(out=agg[:, DX:DX + 1], lhsT=sxw[:],
                                             rhs=ones_c[:],
                                             start=False, stop=False)
                            nc.tensor.matmul(out=agg[:, DX + 1:DX + 1 + DY],
                                             lhsT=syw[:], rhs=gsl[:, DX:DROW],
                                             start=False, stop=False)
                            nc.tensor.matmul(out=agg[:, DROW + 1:DROW + 2],
                                             lhsT=syw[:], rhs=ones_c[:],
                                             start=False, stop=last)

                    # ---- block epilogue ----
                    inv2 = ep.tile([P, 2], f32, tag="inv")
                    nc.vector.tensor_scalar_max(out=inv2[:, 0:1],
                                                in0=agg[:, DX:DX + 1],
                                                scalar1=1e-30)
                    nc.vector.tensor_scalar_max(out=inv2[:, 1:2],
                                                in0=agg[:, DROW + 1:DROW + 2],
                                                scalar1=1e-30)
                    nc.vector.reciprocal(out=inv2[:], in_=inv2[:])
                    oxy = ep.tile([P, 3 * P], f16, tag="oxy")
                    nc.scalar.activation(out=oxy[:, 0:DX], in_=agg[:, 0:DX],
                                         func=mybir.ActivationFunctionType.Copy,
                                         scale=inv2[:, 0:1])
                    nc.scalar.activation(out=oxy[:, DX:DROW],
                                         in_=agg[:, DX + 1:DX + 1 + DY],
                                         func=mybir.ActivationFunctionType.Copy,
                                         scale=inv2[:, 1:2])
                    nc.vector.memset(oxy[:, DROW:3 * P], 0.0)
                    if pas == 0:
                        r2n = ep.tile([P, 1], f32, tag="r2n")
                        jr = jk.tile([P, DX], f16, tag="jr")
                        nc.vector.scalar_tensor_tensor(
                            out=jr[:], in0=oxy[:, 0:DX], scalar=1.0,
                            in1=oxy[:, 0:DX], op0=mybir.AluOpType.mult,
                            op1=mybir.AluOpType.mult, accum_out=r2n[:])
                        nc.vector.tensor_copy(out=oxy[:, DROW:DROW + 1],
                                              in_=r2n[:])
                    if pas == 0:
                        nc.sync.dma_start(out=t1_slice[b * P:(b + 1) * P, :],
                                          in_=oxy[:, 0:TW])

                    # out += feats_level @ W via DMA-transposed chunks
                    if pas == 0:
                        pieces = [(rp[:, 0:P], P, wt[0]), (rp[:, P:DX], P, wt[1]),
                                  (oxy[:, 0:P], P, wt[2]), (oxy[:, P:DX], P, wt[3]),
                                  (oxy[:, DX:3 * P], DY, wy[0])]
                    else:
                        pieces = [(oxy[:, 0:P], P, wt[4]), (oxy[:, P:DX], P, wt[5]),
                                  (oxy[:, DX:3 * P], DY, wy[1])]
                    op = ps_o.tile([P, 64], f32, tag="op")
                    for k, (src, kdim, wchunk) in enumerate(pieces):
                        ft = ep.tile([P, P], f16, tag="ft")
                        nc.sync.dma_start(out=ft[:], in_=src, transpose=True)
                        nc.tensor.matmul(out=op[:], lhsT=ft[0:kdim, :],
                                         rhs=wchunk[:],
                                         start=(k == 0), stop=(k == len(pieces) - 1))
                    if pas == 0:
                        nc.vector.tensor_copy(out=out_acc[:, b * 64:(b + 1) * 64],
                                              in_=op[:])
                        if with_collective and b % NCH == NCH - 1:
                            c = b // NCH
                            nc.gpsimd.collective_compute(
                                "AllGather", mybir.AluOpType.bypass,
                                replica_groups=[list(range(NCORES))],
                                ins=[t1_slice[c * CHROWS:(c + 1) * CHROWS, :].opt()],
                                outs=[t1_full[c * CHFULL:(c + 1) * CHFULL, :].opt()],
                            )
                    else:
                        nc.vector.tensor_add(out=out_acc[:, b * 64:(b + 1) * 64],
                                             in0=out_acc[:, b * 64:(b + 1) * 64],
                                             in1=op[:])
                        nc.sync.dma_start(out=out_d[b * P:(b + 1) * P, :],
                                          in_=out_acc[:, b * 64:(b + 1) * 64])
    nc.compile()
    return nc


def _host_prep(x, y_one_hot, W, sigmas, row, col):
    x = np.asarray(x, dtype=np.float32)
    y_one_hot = np.asarray(y_one_hot, dtype=np.float32)
    W = np.asarray(W, dtype=np.float32)
    sigmas = np.asarray(sigmas, dtype=np.float32)
    row = np.asarray(row, dtype=np.int32)
    col = np.asarray(col, dtype=np.int32)

    bounds = np.searchsorted(row, np.arange(0, N_PAD + 1, P)).astype(np.int64)
    counts = bounds[1:] - bounds[:-1]
    tpb_g = -(-counts // P)
    tpb = np.maximum(tpb_g.reshape(NCORES, BPC).max(axis=0), 1).astype(int)
    off = np.concatenate([[0], np.cumsum(tpb)]).astype(int)
    T_tiles = int(off[-1])

    # chunk-major remap for pass-B gathers out of the AllGathered t1 table
    def remap(n):
        k = n // RPC
        r = n % RPC
        return (r // CHROWS) * CHFULL + k * CHROWS + (r % CHROWS)

    colA = np.zeros((NCORES, P, T_tiles), dtype=np.int32)
    colB = np.zeros((NCORES, P, T_tiles), dtype=np.int32)
    rowl_arr = np.full((NCORES, P, T_tiles), -1.0, dtype=np.float32)
    col_remap = remap(np.arange(N_NODES, dtype=np.int64)).astype(np.int32)
    for c in range(NCORES):
        for b in range(BPC):
            g = c * BPC + b
            e0, e1 = int(bounds[g]), int(bounds[g + 1])
            cnt = e1 - e0
            ntl = int(tpb[b])
            cp = np.zeros(ntl * P, dtype=np.int32)
            rp_ = np.full(ntl * P, -1.0, dtype=np.float32)
            cp[:cnt] = col[e0:e1]
            rp_[:cnt] = (row[e0:e1] - g * P).astype(np.float32)
            colA[c, :, off[b]:off[b] + ntl] = cp.reshape(ntl, P).T
            colB[c, :, off[b]:off[b] + ntl] = col_remap[cp].reshape(ntl, P).T
            rowl_arr[c, :, off[b]:off[b] + ntl] = rp_.reshape(ntl, P).T

    t0 = np.zeros((N_PAD, TW), dtype=np.float16)
    t0[:N_NODES, 0:DX] = x
    t0[:N_NODES, DX:DROW] = y_one_hot
    xf16 = t0[:, 0:DX].astype(np.float32)
    t0[:, DROW] = (xf16 * xf16).sum(axis=1).astype(np.float16)
    iota = np.tile(np.arange(P, dtype=np.float16)[None, :], (P, 1))
    iotaP = np.tile(np.arange(P, dtype=np.float16)[:, None], (1, KM * P))
    negI = (-np.eye(P)).astype(np.float16)

    s2 = sigmas.astype(np.float64) ** 2
    sig_scales = tuple(float(-1.0 / s2[i]) for i in (0, 2, 1, 3))

    in_maps = []
    for c in range(NCORES):
        # rowlT[t, e] = rowl_arr[c][e, t]
        in_maps.append({
            "t0_full": t0,
            "t0_local": t0[c * RPC:(c + 1) * RPC],
            "colrA": np.ascontiguousarray(colA[c]),
            "colrB": np.ascontiguousarray(colB[c]),
            "rowlf": np.ascontiguousarray(rowl_arr[c]),
            "rowlT_d": np.ascontiguousarray(rowl_arr[c].T.astype(np.float16)),
            "iota": iota,
            "iotaP": iotaP,
            "negI": negI,
            "w_mat": W.astype(np.float16),
        })
    return [int(v) for v in tpb], sig_scales, in_maps


def kernel(x, y_one_hot, W, sigmas, row, col):
    global LAST_RESULT, LAST_NC, LAST_IN_MAPS
    from concourse.bass_utils import run_bass_kernel_spmd

    tpb, sig_scales, in_maps = _host_prep(x, y_one_hot, W, sigmas, row, col)
    nc = _build_program(tpb, sig_scales)

    res = run_bass_kernel_spmd(nc, in_maps, core_ids=list(range(NCORES)))
    LAST_RESULT = res
    LAST_NC = nc
    LAST_IN_MAPS = in_maps
    out = np.concatenate([r["out"] for r in res.results], axis=0)
    return np.ascontiguousarray(out[:N_NODES])
